# revision 16
# baseline (speedup 1.0000x reference)
"""Trainium2 Bass kernel for causal GQA self-attention (B=2, S=2048, H=2048,
16 heads / 4 KV heads, head_dim 128) on 8 NeuronCores.

Sharding: core i = (batch b = i//4, head-group g = i%4). Each core computes
QKV for heads 4g..4g+3 (= KV head g, no KV duplication) on its batch's 2048
rows, fused with flash-style attention per 512-row block as soon as that
block's QKV lands. Two 8-way AllToAlls then switch to row-sharding: core j
computes rows 256j..256j+256 of BOTH batches through the output projection
with the full SBUF-resident Wo (each 512-row attention block splits its output
between two destination cores, so the 8-way exchange is fully utilized).

Per-core dataflow: XT[h, r] (bf16, host-pretransposed, batch slice) -> QT/KT
channel-major via weight-stationary matmuls (per-output kt-loops, 2 rotating
PSUM banks); V directly in [k, d] layout via X-stationary matmuls (bias as a
rank-1 ones matmul). scores S^T[k, q] = KT_tile.T @ QT; causal via
compile-time tile skipping + one [128,128] triangle mask added on GpSimd; exp
on ScalarE; AV and a 128-row column-sum matmul (ones stationary - its output
IS the broadcast denominator) accumulate in PSUM; normalize with
reciprocal_approx_fast + one vector multiply. Four late attention blocks are
held back to overlap the first collective.

The attention_mask input is all-ones for this problem (spec fill=ones), so it
is ignored. All matmuls take bf16 inputs (fp32 PSUM accumulate).
"""

import sys

sys.path.insert(0, "/opt/trn_rl_repo")

from contextlib import ExitStack

import numpy as np
import ml_dtypes

import concourse.bass as bass
import concourse.mybir as mybir
import concourse.tile as tile
from concourse import bacc
from concourse.bass_utils import run_bass_kernel_spmd

F32 = mybir.dt.float32
BF16 = mybir.dt.bfloat16
AF = mybir.ActivationFunctionType

N_CORES = 8
B, S, HID = 2, 2048, 2048
NH, NKV, D = 16, 4, 128
SCALE = 1.0 / np.sqrt(D)
NEG = -1e30
P = 128
N_KT = HID // P  # 16 contraction tiles
N_QB = S // 512  # 4 row blocks per core (one batch)
HELD = {(2, 1), (3, 1), (2, 2), (3, 2), (2, 3), (3, 3)}  # run under A2A-1


def build_nc(debug=False):
    nc = bacc.Bacc("TRN2", target_bir_lowering=False, debug=debug, num_devices=8)

    # host-prepacked so every DMA line is >=2KB per partition:
    # xt[p, qb, t, r] ; wq[p, t, c] ; wk/wv[p, t*c]
    xt = nc.dram_tensor("xt", [P, N_QB, N_KT, 512], BF16, kind="ExternalInput")
    wq = nc.dram_tensor("wq", [P, N_KT, 512], BF16, kind="ExternalInput")
    wk = nc.dram_tensor("wk", [P, N_KT * 128], BF16, kind="ExternalInput")
    wv = nc.dram_tensor("wv", [P, N_KT * 128], BF16, kind="ExternalInput")
    bq = nc.dram_tensor("bq", [512, 1], F32, kind="ExternalInput")
    bk = nc.dram_tensor("bk", [128, 1], F32, kind="ExternalInput")
    bvr = nc.dram_tensor("bvr", [1, 128], BF16, kind="ExternalInput")
    wo = nc.dram_tensor("wo", [HID, HID], BF16, kind="ExternalInput")
    bo = nc.dram_tensor("bo", [1, HID], BF16, kind="ExternalInput")
    mtri = nc.dram_tensor("mtri", [P, P], BF16, kind="ExternalInput")
    onesd = nc.dram_tensor("onesd", [P, P], BF16, kind="ExternalInput")
    y = nc.dram_tensor("y", [512, HID], F32, kind="ExternalOutput")


    with tile.TileContext(nc) as tc, ExitStack() as top:
        persist = top.enter_context(tc.tile_pool(name="persist", bufs=1))
        dram = top.enter_context(tc.tile_pool(name="dram", bufs=1, space="DRAM"))

        a2a_in = [dram.tile([8, 2, P, 256], BF16, name=f"a2a_in{h}") for h in range(2)]
        a2a_out = [dram.tile([8, 2, P, 256], BF16, name=f"a2a_out{h}") for h in range(2)]

        wq_sb = persist.tile([P, N_KT, 512], BF16, tag="wq")
        wk_sb = persist.tile([P, N_KT, 128], BF16, tag="wk")
        wv_sb = persist.tile([P, N_KT, 128], BF16, tag="wv")

        # Small consts + the phase-4 Wo stream ride the gpsimd queue.
        ones_sq = persist.tile([P, P], BF16, tag="ones_sq")
        nc.gpsimd.dma_start(ones_sq[:], onesd[:])
        mtri_sb = persist.tile([P, P], BF16, tag="mtri")
        nc.gpsimd.dma_start(mtri_sb[:], mtri[:])
        bq_sb = persist.tile([P, 4], F32, tag="bq")
        for hh in range(4):
            nc.gpsimd.dma_start(bq_sb[:, hh : hh + 1], bq[128 * hh : 128 * (hh + 1), :])
        bk_sb = persist.tile([P, 1], F32, tag="bk")
        nc.gpsimd.dma_start(bk_sb[:], bk[:])
        bvr_sb = persist.tile([1, P], BF16, tag="bvr")
        nc.gpsimd.dma_start(bvr_sb[:], bvr[:])
        bo_sb = persist.tile([1, HID], BF16, tag="bo")
        nc.gpsimd.dma_start(bo_sb[:], bo[:])
        wo_sb = persist.tile([P, N_KT, HID], BF16, tag="wo")
        wo_loaded = [0]  # chunks streamed in during attention (HBM-idle window)

        # channel-major activations: partitions = feature dim
        qt_sb = persist.tile([P, 4, S], BF16, tag="qt")
        kt_sb = persist.tile([P, S], BF16, tag="kt")
        v_sb = persist.tile([P, N_KT, P], BF16, tag="v")  # [krow%128, ktile, d]

        with ExitStack() as body:
            xpool = body.enter_context(tc.tile_pool(name="xp", bufs=2))
            espool = body.enter_context(tc.tile_pool(name="es", bufs=18))
            bcpool = body.enter_context(tc.tile_pool(name="bc", bufs=2))
            aopool = body.enter_context(tc.tile_pool(name="ao", bufs=2))
            ps = body.enter_context(tc.tile_pool(name="ps", bufs=2, space="PSUM"))

            def attn_block(h, qb):
                h2, hj = h // 2, h % 2
                ktiles = list(range(4 * qb, 4 * qb + 4)) + list(range(4 * qb))
                ps_av = ps.tile([P, 512], F32, tag="av", name="ps_av")
                ps_cs = ps.tile([P, 512], F32, tag="cs", bufs=1, name="ps_cs")
                n_kt_q = len(ktiles)

                def emit_av(ki, q0, es, st, sp):
                    nc.tensor.matmul(
                        ps_av[:, q0:512], v_sb[:, ki, :],
                        es[:, q0:512], start=st, stop=sp,
                        skip_group_check=True,
                    )

                pending = []  # software-pipeline AV two k-tiles behind
                cs_args = []  # column-sum matmuls batched at block end: the
                # ones stationary then loads once instead of per strip
                for idx, ki in enumerate(ktiles):
                    diag = ki >= 4 * qb
                    q0 = 128 * ki - 512 * qb if diag else 0
                    ps_s = ps.tile([P, 512], F32, tag="s", bufs=3, name="ps_s")
                    ksl = kt_sb[:, P * ki : P * (ki + 1)]
                    qsl = qt_sb[:, h, 512 * qb + q0 : 512 * (qb + 1)]
                    nc.tensor.matmul(ps_s[:, q0:512], ksl, qsl, start=True, stop=True)
                    es = espool.tile([P, 512], BF16, tag="es", name="es")
                    nc.scalar.activation(
                        es[:, q0:512], ps_s[:, q0:512], AF.Exp, scale=SCALE
                    )
                    if diag:
                        nc.vector.tensor_mul(
                            es[:, q0 : q0 + P], es[:, q0 : q0 + P], mtri_sb[:]
                        )
                    if len(pending) == 2:
                        emit_av(*pending.pop(0))
                    pending.append((ki, q0, es, idx == 0, idx == n_kt_q - 1))
                    cs_args.append((q0, es))
                for args in pending:
                    emit_av(*args)
                for idx, (q0, es) in enumerate(cs_args):
                    nc.tensor.matmul(
                        ps_cs[:, q0:512], ones_sq[:, :],
                        es[:, q0:512], start=(idx == 0), stop=(idx == n_kt_q - 1),
                        skip_group_check=True,
                    )

                bc = bcpool.tile([P, 512], F32, tag="bc", name="bc")
                nc.vector.reciprocal_approx_fast(out=bc[:], in_=ps_cs[:])
                ao = aopool.tile([P, 512], BF16, tag="ao", name="ao")
                nc.vector.tensor_mul(ao[:], ps_av[:], bc[:])
                nc.sync.dma_start(a2a_in[h2][2 * qb, hj, :, :], ao[:, 0:256])
                nc.sync.dma_start(a2a_in[h2][2 * qb + 1, hj, :, :], ao[:, 256:512])
                # 2 Wo chunks ride behind each early block's ao DMA: the ao's
                # wait on this block's output keeps the 8MB Wo stream out of
                # the phase-1 xt window (SWDGE DMAs have no data deps of their
                # own and would otherwise all fire at t=0)
                for _ in range(2):
                    if wo_loaded[0] < N_KT:
                        t = wo_loaded[0]
                        nc.sync.dma_start(wo_sb[:, t, :], wo[P * t : P * (t + 1), :])
                        wo_loaded[0] += 1

            # ---- fused QKV projection + attention, per 512-row block ----
            def load_xt(qb):
                xt_t = xpool.tile([P, N_KT, 512], BF16, tag="x", name="xt_t")
                for kc in range(4):
                    if qb == 0 and kc == 0:
                        nc.sync.dma_start(wk_sb[:].rearrange("p t c -> p (t c)"), wk[:, :])
                        nc.sync.dma_start(wv_sb[:].rearrange("p t c -> p (t c)"), wv[:, :])
                    nc.sync.dma_start(
                        xt_t[:, 4 * kc : 4 * (kc + 1), :],
                        xt[:, qb, 4 * kc : 4 * (kc + 1), :],
                    )
                    if qb == 0:
                        nc.sync.dma_start(
                            wq_sb[:, 4 * kc : 4 * kc + 4, :], wq[:, 4 * kc : 4 * kc + 4, :]
                        )
                return xt_t

            # HAM pre-warm: ~40 tiny matmuls keep the PE busy through its
            # 3.4us activity window while the first weights/xt stream in, so
            # the first real matmuls run at 2.4GHz instead of 1.2
            warm_ps = ps.tile([P, 512], F32, tag="p1", name="warm_ps")
            for wi in range(200):
                nc.tensor.matmul(
                    warm_ps[:, 0:128], ones_sq[:, :], ones_sq[:, :],
                    start=(wi == 0), stop=(wi == 199), skip_group_check=True,
                )
            xt_next = load_xt(0)
            for qb in range(N_QB):
                rsl = slice(512 * qb, 512 * (qb + 1))
                xt_t = xt_next
                # K: channel-major, weight-stationary
                ps_k = ps.tile([P, 512], F32, tag="p1", name="ps_k")
                for kt_i in range(N_KT):
                    nc.tensor.matmul(
                        ps_k[:], wk_sb[:, kt_i, :], xt_t[:, kt_i, :],
                        start=(kt_i == 0), stop=(kt_i == N_KT - 1),
                    )
                nc.vector.tensor_scalar_add(kt_sb[:, rsl], ps_k[:], bk_sb[:])
                # V: [k, d] layout, X-stationary; bias via rank-1 ones matmul
                ps_v = ps.tile([P, 4, P], F32, tag="p1", name="ps_v")
                for kb in range(4):
                    for kt_i in range(N_KT):
                        nc.tensor.matmul(
                            ps_v[:, kb, :],
                            xt_t[:, kt_i, P * kb : P * (kb + 1)],
                            wv_sb[:, kt_i, :],
                            start=(kt_i == 0), stop=False,
                            skip_group_check=True,
                        )
                    nc.tensor.matmul(
                        ps_v[:, kb, :], ones_sq[0:1, :], bvr_sb[:],
                        start=False, stop=True, skip_group_check=True,
                    )
                nc.vector.tensor_copy(v_sb[:, 4 * qb : 4 * qb + 4, :], ps_v[:])
                # Q per head, each head's attention block right behind it
                for hh in range(4):
                    ps_q = ps.tile([P, 512], F32, tag="p1", name="ps_q")
                    for kt_i in range(N_KT):
                        nc.tensor.matmul(
                            ps_q[:], wq_sb[:, kt_i, P * hh : P * (hh + 1)],
                            xt_t[:, kt_i, :],
                            start=(kt_i == 0), stop=(kt_i == N_KT - 1),
                        )
                    nc.vector.tensor_scalar_add(
                        qt_sb[:, hh, rsl], ps_q[:], bq_sb[:, hh : hh + 1]
                    )
                    if hh == 0 and qb + 1 < N_QB:
                        # prefetch the next row block before this qb's ao DMAs
                        # can head-of-line-block the sync queue
                        xt_next = load_xt(qb + 1)
                    if (hh, qb) not in HELD:
                        attn_block(hh, qb)

            # ---- A2A for heads {0,1} of each peer; held blocks run under it
            nc.gpsimd.collective_compute(
                "AllToAll",
                mybir.AluOpType.bypass,
                replica_groups=[list(range(N_CORES))],
                ins=[a2a_in[0][:]],
                outs=[a2a_out[0][:]],
            )
            for h, qb in sorted(HELD, key=lambda t: (t[1], t[0])):
                attn_block(h, qb)
            nc.gpsimd.collective_compute(
                "AllToAll",
                mybir.AluOpType.bypass,
                replica_groups=[list(range(N_CORES))],
                ins=[a2a_in[1][:]],
                outs=[a2a_out[1][:]],
            )

        # ---- o_proj (512 rows x 2048, SBUF-resident Wo) ----
        # hd-tile t = head t (channels 128t..); t%4 in {0,1} arrives with
        # A2A-1, {2,3} with A2A-2.
        with ExitStack() as ph4:
            atpool = ph4.enter_context(tc.tile_pool(name="at", bufs=1))
            ypool = ph4.enter_context(tc.tile_pool(name="yp", bufs=4))
            yppool = ph4.enter_context(tc.tile_pool(name="ypart", bufs=1))
            pso = ph4.enter_context(tc.tile_pool(name="pso", bufs=8, space="PSUM"))
            pass1 = [t for t in range(N_KT) if t % 4 < 2]
            pass2 = [t for t in range(N_KT) if t % 4 >= 2]
            # head t = 4g + 2*h2 + hj comes from cores g (batch-0 row half)
            # and 4+g (batch-1 half). One DMA per A2A half: dst free dims
            # (g, b, hj, c) <- src dims (b, g, hj, p, c).
            at_all = [None, None]
            for h2 in range(2):
                a = atpool.tile([P, 4, 2, 2, 256], BF16, tag=f"atall{h2}", name=f"atall{h2}")
                for bb in range(2):
                    nc.gpsimd.dma_start(
                        a[:, :, :, bb, :],
                        a2a_out[h2][4 * bb : 4 * bb + 4, :, :, :].rearrange(
                            "g hj p c -> p g hj c"
                        ),
                    )
                at_all[h2] = a
            # at[t] view [p, 512]: head t rows = [b0 256 | b1 256]
            at = [
                at_all[(t % 4) // 2][:, t // 4, t % 2, :, :].rearrange("p b c -> p (b c)")
                for t in range(N_KT)
            ]
            ypart = [[None] * 4 for _ in range(4)]
            for nbp in range(2):  # nb pairs share each stationary LDWEIGHTS
                nbs = (2 * nbp, 2 * nbp + 1)
                ps_os = {
                    (nb, q): pso.tile([P, 512], F32, tag="po", name=f"ps_o{nb}_{q}")
                    for nb in nbs for q in range(4)
                }
                for ti, t in enumerate(pass1):
                    for qt_i in range(4):
                        for nb in nbs:
                            nc.tensor.matmul(
                                ps_os[nb, qt_i][:], at[t][:, P * qt_i : P * (qt_i + 1)],
                                wo_sb[:, t, 512 * nb : 512 * (nb + 1)],
                                start=(ti == 0), stop=False,
                                skip_group_check=True,
                            )
                for nb in nbs:
                    for qt_i in range(4):
                        nc.tensor.matmul(
                            ps_os[nb, qt_i][:], ones_sq[0:1, :],
                            bo_sb[0:1, 512 * nb : 512 * (nb + 1)], start=False, stop=True,
                            skip_group_check=True,
                        )
                        yp = yppool.tile([P, 512], F32, tag=f"yp{nb}_{qt_i}", name="yp")
                        nc.vector.tensor_copy(yp[:], ps_os[nb, qt_i][:])
                        ypart[nb][qt_i] = yp
            for nbp in range(2):
                nbs = (2 * nbp, 2 * nbp + 1)
                last = nbp == 1
                if not last:
                    ps_o2 = {
                        (nb, q): pso.tile([P, 512], F32, tag="po", name=f"ps_p{nb}_{q}")
                        for nb in nbs for q in range(4)
                    }
                    for ti, t in enumerate(pass2):
                        for qt_i in range(4):
                            for nb in nbs:
                                nc.tensor.matmul(
                                    ps_o2[nb, qt_i][:], at[t][:, P * qt_i : P * (qt_i + 1)],
                                    wo_sb[:, t, 512 * nb : 512 * (nb + 1)],
                                    start=(ti == 0), stop=(ti == len(pass2) - 1),
                                    skip_group_check=True,
                                )
                    for nb in nbs:
                        nsl = slice(512 * nb, 512 * (nb + 1))
                        for qt_i in range(4):
                            ysb = ypool.tile([P, 512], F32, tag="y", name="ysb")
                            nc.vector.tensor_add(ysb[:], ps_o2[nb, qt_i][:], ypart[nb][qt_i][:])
                            nc.sync.dma_start(y[P * qt_i : P * (qt_i + 1), nsl], ysb[:])
                else:
                    # last pair: (nb, qt) outer so each bank finishes early and
                    # its eviction + y DMA overlap the remaining matmuls
                    for nb in nbs:
                        nsl = slice(512 * nb, 512 * (nb + 1))
                        for qt_i in range(4):
                            ps_p = pso.tile([P, 512], F32, tag="po", name="ps_p")
                            for ti, t in enumerate(pass2):
                                nc.tensor.matmul(
                                    ps_p[:], at[t][:, P * qt_i : P * (qt_i + 1)],
                                    wo_sb[:, t, 512 * nb : 512 * (nb + 1)],
                                    start=(ti == 0), stop=(ti == len(pass2) - 1),
                                    skip_group_check=True,
                                )
                            ysb = ypool.tile([P, 512], F32, tag="y", name="ysb")
                            nc.vector.tensor_add(ysb[:], ps_p[:], ypart[nb][qt_i][:])
                            nc.sync.dma_start(y[P * qt_i : P * (qt_i + 1), nsl], ysb[:])

    nc.compile()
    return nc


def make_in_maps(hidden_states, Wq, bq, Wk, bk, Wv, bv, Wo, bo):
    X = np.asarray(hidden_states, np.float32)  # [B, S, HID]
    qq = np.arange(P)[None, :]
    kk = np.arange(P)[:, None]
    mtri = np.where(qq >= kk, 1.0, 0.0).astype(ml_dtypes.bfloat16)
    Wq = np.asarray(Wq, np.float32)
    Wk = np.asarray(Wk, np.float32)
    Wv = np.asarray(Wv, np.float32)
    Wo = np.ascontiguousarray(np.asarray(Wo, np.float32)).astype(ml_dtypes.bfloat16)
    bq = np.asarray(bq, np.float32)
    bk = np.asarray(bk, np.float32)
    bv = np.asarray(bv, np.float32)
    bo = np.asarray(bo, np.float32)
    def pack_w(W):  # [HID, C] -> [128, N_KT, C] (t = hid tile)
        return np.ascontiguousarray(
            W.reshape(N_KT, P, -1).transpose(1, 0, 2)
        ).astype(ml_dtypes.bfloat16)

    xts = []
    for b in range(B):
        XT = X[b].T.reshape(N_KT, P, N_QB, 512)  # [t, p, qb, r]
        xts.append(
            np.ascontiguousarray(XT.transpose(1, 2, 0, 3)).astype(ml_dtypes.bfloat16)
        )
    in_maps = []
    for i in range(N_CORES):
        b, g = i // 4, i % 4
        in_maps.append({
            "xt": xts[b],
            "wq": pack_w(Wq[:, 512 * g : 512 * (g + 1)]),
            "wk": pack_w(Wk[:, 128 * g : 128 * (g + 1)]).reshape(P, N_KT * 128),
            "wv": pack_w(Wv[:, 128 * g : 128 * (g + 1)]).reshape(P, N_KT * 128),
            "bq": np.ascontiguousarray(bq[512 * g : 512 * (g + 1)]).reshape(512, 1),
            "bk": np.ascontiguousarray(bk[128 * g : 128 * (g + 1)]).reshape(128, 1),
            "bvr": np.ascontiguousarray(bv[128 * g : 128 * (g + 1)]).reshape(1, 128).astype(ml_dtypes.bfloat16),
            "wo": Wo,
            "bo": bo.reshape(1, HID).astype(ml_dtypes.bfloat16),
            "mtri": mtri,
            "onesd": np.ones((P, P), ml_dtypes.bfloat16),
        })
    return in_maps


def assemble(results):
    Y = np.empty((B, S, HID), np.float32)
    for j in range(N_CORES):
        Y[0, 256 * j : 256 * (j + 1), :] = results[j]["y"][0:256]
        Y[1, 256 * j : 256 * (j + 1), :] = results[j]["y"][256:512]
    return Y


_NC_CACHE = {}


def _get_nc(debug=False):
    if debug not in _NC_CACHE:
        _NC_CACHE[debug] = build_nc(debug=debug)
    return _NC_CACHE[debug]


def kernel(hidden_states, attention_mask, Wq, bq, Wk, bk, Wv, bv, Wo, bo):
    # attention_mask is all-ones for this problem (spec: fill=ones) -> ignored
    nc = _get_nc(debug=False)
    in_maps = make_in_maps(hidden_states, Wq, bq, Wk, bk, Wv, bv, Wo, bo)
    res = run_bass_kernel_spmd(nc, in_maps, core_ids=list(range(N_CORES)))
    return assemble(res.results)


# revision 18
# speedup vs baseline: 1.1580x; 1.1580x over previous
"""Trainium2 Bass kernel for causal GQA self-attention (B=2, S=2048, H=2048,
16 heads / 4 KV heads, head_dim 128) on 8 NeuronCores.

Sharding: core i = (batch b = i//4, head-group g = i%4). Each core computes
QKV for heads 4g..4g+3 (= KV head g, no KV duplication) on its batch's 2048
rows, fused with flash-style attention per 512-row block as soon as that
block's QKV lands. Two 8-way AllToAlls then switch to row-sharding: core j
computes rows 256j..256j+256 of BOTH batches through the output projection
with the full SBUF-resident Wo (each 512-row attention block splits its output
between two destination cores, so the 8-way exchange is fully utilized).

Per-core dataflow: XT[h, r] (bf16, host-pretransposed, batch slice) -> QT/KT
channel-major via weight-stationary matmuls (per-output kt-loops, 2 rotating
PSUM banks); V directly in [k, d] layout via X-stationary matmuls (bias as a
rank-1 ones matmul). scores S^T[k, q] = KT_tile.T @ QT; causal via
compile-time tile skipping + one [128,128] triangle mask added on GpSimd; exp
on ScalarE; AV and a 128-row column-sum matmul (ones stationary - its output
IS the broadcast denominator) accumulate in PSUM; normalize with
reciprocal_approx_fast + one vector multiply. Four late attention blocks are
held back to overlap the first collective.

The attention_mask input is all-ones for this problem (spec fill=ones), so it
is ignored. All matmuls take bf16 inputs (fp32 PSUM accumulate).
"""

import sys

sys.path.insert(0, "/opt/trn_rl_repo")

from contextlib import ExitStack

import numpy as np
import ml_dtypes

import concourse.bass as bass
import concourse.mybir as mybir
import concourse.tile as tile
from concourse import bacc
from concourse.bass_utils import run_bass_kernel_spmd

F32 = mybir.dt.float32
BF16 = mybir.dt.bfloat16
AF = mybir.ActivationFunctionType

N_CORES = 8
B, S, HID = 2, 2048, 2048
NH, NKV, D = 16, 4, 128
SCALE = 1.0 / np.sqrt(D)
NEG = -1e30
P = 128
N_KT = HID // P  # 16 contraction tiles
N_QB = S // 512  # 4 row blocks per core (one batch)
HELD = {(2, 1), (3, 1), (2, 2), (3, 2), (2, 3), (3, 3)}  # run under A2A-1


def build_nc(debug=False):
    nc = bacc.Bacc("TRN2", target_bir_lowering=False, debug=debug, num_devices=8)

    # host-prepacked so every DMA line is >=2KB per partition:
    # xt[p, qb, t, r] ; wq[p, t, c] ; wk/wv[p, t*c]
    xt = nc.dram_tensor("xt", [P, N_QB, N_KT, 512], BF16, kind="ExternalInput")
    wq = nc.dram_tensor("wq", [P, N_KT, 512], BF16, kind="ExternalInput")
    wk = nc.dram_tensor("wk", [P, N_KT * 128], BF16, kind="ExternalInput")
    wv = nc.dram_tensor("wv", [P, N_KT * 128], BF16, kind="ExternalInput")
    bq = nc.dram_tensor("bq", [512, 1], F32, kind="ExternalInput")
    bk = nc.dram_tensor("bk", [128, 1], F32, kind="ExternalInput")
    bvr = nc.dram_tensor("bvr", [1, 128], BF16, kind="ExternalInput")
    wo = nc.dram_tensor("wo", [HID, HID], BF16, kind="ExternalInput")
    bo = nc.dram_tensor("bo", [1, HID], BF16, kind="ExternalInput")
    mtri = nc.dram_tensor("mtri", [P, P], BF16, kind="ExternalInput")
    onesd = nc.dram_tensor("onesd", [P, P], BF16, kind="ExternalInput")
    y = nc.dram_tensor("y", [512, HID], F32, kind="ExternalOutput")


    with tile.TileContext(nc) as tc, ExitStack() as top:
        persist = top.enter_context(tc.tile_pool(name="persist", bufs=1))
        dram = top.enter_context(tc.tile_pool(name="dram", bufs=1, space="DRAM"))

        a2a_in = [dram.tile([8, 2, P, 256], BF16, name=f"a2a_in{h}") for h in range(2)]
        a2a_out = [dram.tile([8, 2, P, 256], BF16, name=f"a2a_out{h}") for h in range(2)]

        wq_sb = persist.tile([P, N_KT, 512], BF16, tag="wq")
        wk_sb = persist.tile([P, N_KT, 128], BF16, tag="wk")
        wv_sb = persist.tile([P, N_KT, 128], BF16, tag="wv")

        # Small consts + the phase-4 Wo stream ride the gpsimd queue.
        ones_sq = persist.tile([P, P], BF16, tag="ones_sq")
        nc.gpsimd.dma_start(ones_sq[:], onesd[:])
        mtri_sb = persist.tile([P, P], BF16, tag="mtri")
        nc.gpsimd.dma_start(mtri_sb[:], mtri[:])
        bq_sb = persist.tile([P, 4], F32, tag="bq")
        for hh in range(4):
            nc.gpsimd.dma_start(bq_sb[:, hh : hh + 1], bq[128 * hh : 128 * (hh + 1), :])
        bk_sb = persist.tile([P, 1], F32, tag="bk")
        nc.gpsimd.dma_start(bk_sb[:], bk[:])
        bvr_sb = persist.tile([1, P], BF16, tag="bvr")
        nc.gpsimd.dma_start(bvr_sb[:], bvr[:])
        bo_sb = persist.tile([1, HID], BF16, tag="bo")
        nc.gpsimd.dma_start(bo_sb[:], bo[:])
        wo_sb = persist.tile([P, N_KT, HID], BF16, tag="wo")
        wo_loaded = [0]  # chunks streamed in during attention (HBM-idle window)

        # channel-major activations: partitions = feature dim
        qt_sb = persist.tile([P, 4, S], BF16, tag="qt")
        kt_sb = persist.tile([P, S], BF16, tag="kt")
        v_sb = persist.tile([P, N_KT, P], BF16, tag="v")  # [krow%128, ktile, d]

        with ExitStack() as body:
            xpool = body.enter_context(tc.tile_pool(name="xp", bufs=2))
            espool = body.enter_context(tc.tile_pool(name="es", bufs=6))
            bcpool = body.enter_context(tc.tile_pool(name="bc", bufs=2))
            aopool = body.enter_context(tc.tile_pool(name="ao", bufs=2))
            ps = body.enter_context(tc.tile_pool(name="ps", bufs=2, space="PSUM"))

            def attn_block(h, qb):
                h2, hj = h // 2, h % 2
                ktiles = list(range(4 * qb, 4 * qb + 4)) + list(range(4 * qb))
                ps_av = ps.tile([P, 512], F32, tag="av", name="ps_av")
                ps_cs = ps.tile([P, 512], F32, tag="cs", bufs=1, name="ps_cs")
                n_kt_q = len(ktiles)

                def emit_av(ki, q0, es, st, sp):
                    nc.tensor.matmul(
                        ps_av[:, q0:512], v_sb[:, ki, :],
                        es[:, q0:512], start=st, stop=sp,
                        skip_group_check=True,
                    )
                    nc.tensor.matmul(
                        ps_cs[:, q0:512], ones_sq[:, :],
                        es[:, q0:512], start=st, stop=sp,
                        skip_group_check=True,
                    )

                pending = []  # software-pipeline AV two k-tiles behind
                for idx, ki in enumerate(ktiles):
                    diag = ki >= 4 * qb
                    q0 = 128 * ki - 512 * qb if diag else 0
                    ps_s = ps.tile([P, 512], F32, tag="s", bufs=3, name="ps_s")
                    ksl = kt_sb[:, P * ki : P * (ki + 1)]
                    qsl = qt_sb[:, h, 512 * qb + q0 : 512 * (qb + 1)]
                    nc.tensor.matmul(ps_s[:, q0:512], ksl, qsl, start=True, stop=True)
                    es = espool.tile([P, 512], BF16, tag="es", name="es")
                    nc.scalar.activation(
                        es[:, q0:512], ps_s[:, q0:512], AF.Exp, scale=SCALE
                    )
                    if diag:
                        nc.vector.tensor_mul(
                            es[:, q0 : q0 + P], es[:, q0 : q0 + P], mtri_sb[:]
                        )
                    if len(pending) == 2:
                        emit_av(*pending.pop(0))
                    pending.append((ki, q0, es, idx == 0, idx == n_kt_q - 1))
                for args in pending:
                    emit_av(*args)

                bc = bcpool.tile([P, 512], F32, tag="bc", name="bc")
                nc.vector.reciprocal_approx_fast(out=bc[:], in_=ps_cs[:])
                ao = aopool.tile([P, 512], BF16, tag="ao", name="ao")
                nc.vector.tensor_mul(ao[:], ps_av[:], bc[:])
                nc.sync.dma_start(a2a_in[h2][2 * qb, hj, :, :], ao[:, 0:256])
                nc.sync.dma_start(a2a_in[h2][2 * qb + 1, hj, :, :], ao[:, 256:512])
                # 2 Wo chunks ride behind each early block's ao DMA: the ao's
                # wait on this block's output keeps the 8MB Wo stream out of
                # the phase-1 xt window (SWDGE DMAs have no data deps of their
                # own and would otherwise all fire at t=0)
                for _ in range(2):
                    if wo_loaded[0] < N_KT:
                        t = wo_loaded[0]
                        nc.sync.dma_start(wo_sb[:, t, :], wo[P * t : P * (t + 1), :])
                        wo_loaded[0] += 1

            # ---- fused QKV projection + attention, per 512-row block ----
            def load_xt(qb):
                xt_t = xpool.tile([P, N_KT, 512], BF16, tag="x", name="xt_t")
                for kc in range(4):
                    if qb == 0 and kc == 0:
                        nc.sync.dma_start(wk_sb[:].rearrange("p t c -> p (t c)"), wk[:, :])
                        nc.sync.dma_start(wv_sb[:].rearrange("p t c -> p (t c)"), wv[:, :])
                    nc.sync.dma_start(
                        xt_t[:, 4 * kc : 4 * (kc + 1), :],
                        xt[:, qb, 4 * kc : 4 * (kc + 1), :],
                    )
                    if qb == 0:
                        nc.sync.dma_start(
                            wq_sb[:, 4 * kc : 4 * kc + 4, :], wq[:, 4 * kc : 4 * kc + 4, :]
                        )
                return xt_t

            # HAM pre-warm: ~40 tiny matmuls keep the PE busy through its
            # 3.4us activity window while the first weights/xt stream in, so
            # the first real matmuls run at 2.4GHz instead of 1.2
            warm_ps = ps.tile([P, 512], F32, tag="p1", name="warm_ps")
            for wi in range(280):
                nc.tensor.matmul(
                    warm_ps[:, 0:128], ones_sq[:, :], ones_sq[:, :],
                    start=(wi == 0), stop=(wi == 279), skip_group_check=True,
                )
            xt_next = load_xt(0)
            for qb in range(N_QB):
                rsl = slice(512 * qb, 512 * (qb + 1))
                xt_t = xt_next
                # K: channel-major, weight-stationary
                ps_k = ps.tile([P, 512], F32, tag="p1", name="ps_k")
                for kt_i in range(N_KT):
                    nc.tensor.matmul(
                        ps_k[:], wk_sb[:, kt_i, :], xt_t[:, kt_i, :],
                        start=(kt_i == 0), stop=(kt_i == N_KT - 1),
                    )
                nc.vector.tensor_scalar_add(kt_sb[:, rsl], ps_k[:], bk_sb[:])
                # V: [k, d] layout, X-stationary; bias via rank-1 ones matmul
                ps_v = ps.tile([P, 4, P], F32, tag="p1", name="ps_v")
                for kb in range(4):
                    for kt_i in range(N_KT):
                        nc.tensor.matmul(
                            ps_v[:, kb, :],
                            xt_t[:, kt_i, P * kb : P * (kb + 1)],
                            wv_sb[:, kt_i, :],
                            start=(kt_i == 0), stop=False,
                            skip_group_check=True,
                        )
                    nc.tensor.matmul(
                        ps_v[:, kb, :], ones_sq[0:1, :], bvr_sb[:],
                        start=False, stop=True, skip_group_check=True,
                    )
                nc.vector.tensor_copy(v_sb[:, 4 * qb : 4 * qb + 4, :], ps_v[:])
                # Q per head, each head's attention block right behind it
                for hh in range(4):
                    ps_q = ps.tile([P, 512], F32, tag="p1", name="ps_q")
                    for kt_i in range(N_KT):
                        nc.tensor.matmul(
                            ps_q[:], wq_sb[:, kt_i, P * hh : P * (hh + 1)],
                            xt_t[:, kt_i, :],
                            start=(kt_i == 0), stop=(kt_i == N_KT - 1),
                        )
                    nc.vector.tensor_scalar_add(
                        qt_sb[:, hh, rsl], ps_q[:], bq_sb[:, hh : hh + 1]
                    )
                    if hh == 0 and qb + 1 < N_QB:
                        # prefetch the next row block before this qb's ao DMAs
                        # can head-of-line-block the sync queue
                        xt_next = load_xt(qb + 1)
                    if (hh, qb) not in HELD:
                        attn_block(hh, qb)

            # ---- A2A for heads {0,1} of each peer; held blocks run under it
            nc.gpsimd.collective_compute(
                "AllToAll",
                mybir.AluOpType.bypass,
                replica_groups=[list(range(N_CORES))],
                ins=[a2a_in[0][:]],
                outs=[a2a_out[0][:]],
            )
            for h, qb in sorted(HELD, key=lambda t: (t[1], t[0])):
                attn_block(h, qb)
            nc.gpsimd.collective_compute(
                "AllToAll",
                mybir.AluOpType.bypass,
                replica_groups=[list(range(N_CORES))],
                ins=[a2a_in[1][:]],
                outs=[a2a_out[1][:]],
            )

        # ---- o_proj (512 rows x 2048, SBUF-resident Wo) ----
        # hd-tile t = head t (channels 128t..); t%4 in {0,1} arrives with
        # A2A-1, {2,3} with A2A-2.
        with ExitStack() as ph4:
            atpool = ph4.enter_context(tc.tile_pool(name="at", bufs=1))
            ypool = ph4.enter_context(tc.tile_pool(name="yp", bufs=4))
            yppool = ph4.enter_context(tc.tile_pool(name="ypart", bufs=1))
            pso = ph4.enter_context(tc.tile_pool(name="pso", bufs=8, space="PSUM"))
            pass1 = [t for t in range(N_KT) if t % 4 < 2]
            pass2 = [t for t in range(N_KT) if t % 4 >= 2]
            # head t = 4g + 2*h2 + hj comes from cores g (batch-0 row half)
            # and 4+g (batch-1 half). One DMA per A2A half: dst free dims
            # (g, b, hj, c) <- src dims (b, g, hj, p, c).
            at_all = [None, None]
            for h2 in range(2):
                a = atpool.tile([P, 4, 2, 2, 256], BF16, tag=f"atall{h2}", name=f"atall{h2}")
                for bb in range(2):
                    nc.gpsimd.dma_start(
                        a[:, :, :, bb, :],
                        a2a_out[h2][4 * bb : 4 * bb + 4, :, :, :].rearrange(
                            "g hj p c -> p g hj c"
                        ),
                    )
                at_all[h2] = a
            # at[t] view [p, 512]: head t rows = [b0 256 | b1 256]
            at = [
                at_all[(t % 4) // 2][:, t // 4, t % 2, :, :].rearrange("p b c -> p (b c)")
                for t in range(N_KT)
            ]
            ypart = [[None] * 4 for _ in range(4)]
            for nbp in range(2):  # nb pairs share each stationary LDWEIGHTS
                nbs = (2 * nbp, 2 * nbp + 1)
                ps_os = {
                    (nb, q): pso.tile([P, 512], F32, tag="po", name=f"ps_o{nb}_{q}")
                    for nb in nbs for q in range(4)
                }
                for ti, t in enumerate(pass1):
                    for qt_i in range(4):
                        for nb in nbs:
                            nc.tensor.matmul(
                                ps_os[nb, qt_i][:], at[t][:, P * qt_i : P * (qt_i + 1)],
                                wo_sb[:, t, 512 * nb : 512 * (nb + 1)],
                                start=(ti == 0), stop=False,
                                skip_group_check=True,
                            )
                for nb in nbs:
                    for qt_i in range(4):
                        nc.tensor.matmul(
                            ps_os[nb, qt_i][:], ones_sq[0:1, :],
                            bo_sb[0:1, 512 * nb : 512 * (nb + 1)], start=False, stop=True,
                            skip_group_check=True,
                        )
                        yp = yppool.tile([P, 512], F32, tag=f"yp{nb}_{qt_i}", name="yp")
                        nc.vector.tensor_copy(yp[:], ps_os[nb, qt_i][:])
                        ypart[nb][qt_i] = yp
            for nbp in range(2):
                nbs = (2 * nbp, 2 * nbp + 1)
                last = nbp == 1
                if not last:
                    ps_o2 = {
                        (nb, q): pso.tile([P, 512], F32, tag="po", name=f"ps_p{nb}_{q}")
                        for nb in nbs for q in range(4)
                    }
                    for ti, t in enumerate(pass2):
                        for qt_i in range(4):
                            for nb in nbs:
                                nc.tensor.matmul(
                                    ps_o2[nb, qt_i][:], at[t][:, P * qt_i : P * (qt_i + 1)],
                                    wo_sb[:, t, 512 * nb : 512 * (nb + 1)],
                                    start=(ti == 0), stop=(ti == len(pass2) - 1),
                                    skip_group_check=True,
                                )
                    for nb in nbs:
                        nsl = slice(512 * nb, 512 * (nb + 1))
                        for qt_i in range(4):
                            ysb = ypool.tile([P, 512], F32, tag="y", name="ysb")
                            nc.vector.tensor_add(ysb[:], ps_o2[nb, qt_i][:], ypart[nb][qt_i][:])
                            nc.sync.dma_start(y[P * qt_i : P * (qt_i + 1), nsl], ysb[:])
                else:
                    # last pair: (nb, qt) outer so each bank finishes early and
                    # its eviction + y DMA overlap the remaining matmuls
                    for nb in nbs:
                        nsl = slice(512 * nb, 512 * (nb + 1))
                        for qt_i in range(4):
                            ps_p = pso.tile([P, 512], F32, tag="po", name="ps_p")
                            for ti, t in enumerate(pass2):
                                nc.tensor.matmul(
                                    ps_p[:], at[t][:, P * qt_i : P * (qt_i + 1)],
                                    wo_sb[:, t, 512 * nb : 512 * (nb + 1)],
                                    start=(ti == 0), stop=(ti == len(pass2) - 1),
                                    skip_group_check=True,
                                )
                            ysb = ypool.tile([P, 512], F32, tag="y", name="ysb")
                            nc.vector.tensor_add(ysb[:], ps_p[:], ypart[nb][qt_i][:])
                            nc.sync.dma_start(y[P * qt_i : P * (qt_i + 1), nsl], ysb[:])

    nc.compile()
    return nc


def make_in_maps(hidden_states, Wq, bq, Wk, bk, Wv, bv, Wo, bo):
    X = np.asarray(hidden_states, np.float32)  # [B, S, HID]
    qq = np.arange(P)[None, :]
    kk = np.arange(P)[:, None]
    mtri = np.where(qq >= kk, 1.0, 0.0).astype(ml_dtypes.bfloat16)
    Wq = np.asarray(Wq, np.float32)
    Wk = np.asarray(Wk, np.float32)
    Wv = np.asarray(Wv, np.float32)
    Wo = np.ascontiguousarray(np.asarray(Wo, np.float32)).astype(ml_dtypes.bfloat16)
    bq = np.asarray(bq, np.float32)
    bk = np.asarray(bk, np.float32)
    bv = np.asarray(bv, np.float32)
    bo = np.asarray(bo, np.float32)
    def pack_w(W):  # [HID, C] -> [128, N_KT, C] (t = hid tile)
        return np.ascontiguousarray(
            W.reshape(N_KT, P, -1).transpose(1, 0, 2)
        ).astype(ml_dtypes.bfloat16)

    xts = []
    for b in range(B):
        XT = X[b].T.reshape(N_KT, P, N_QB, 512)  # [t, p, qb, r]
        xts.append(
            np.ascontiguousarray(XT.transpose(1, 2, 0, 3)).astype(ml_dtypes.bfloat16)
        )
    in_maps = []
    for i in range(N_CORES):
        b, g = i // 4, i % 4
        in_maps.append({
            "xt": xts[b],
            "wq": pack_w(Wq[:, 512 * g : 512 * (g + 1)]),
            "wk": pack_w(Wk[:, 128 * g : 128 * (g + 1)]).reshape(P, N_KT * 128),
            "wv": pack_w(Wv[:, 128 * g : 128 * (g + 1)]).reshape(P, N_KT * 128),
            "bq": np.ascontiguousarray(bq[512 * g : 512 * (g + 1)]).reshape(512, 1),
            "bk": np.ascontiguousarray(bk[128 * g : 128 * (g + 1)]).reshape(128, 1),
            "bvr": np.ascontiguousarray(bv[128 * g : 128 * (g + 1)]).reshape(1, 128).astype(ml_dtypes.bfloat16),
            "wo": Wo,
            "bo": bo.reshape(1, HID).astype(ml_dtypes.bfloat16),
            "mtri": mtri,
            "onesd": np.ones((P, P), ml_dtypes.bfloat16),
        })
    return in_maps


def assemble(results):
    Y = np.empty((B, S, HID), np.float32)
    for j in range(N_CORES):
        Y[0, 256 * j : 256 * (j + 1), :] = results[j]["y"][0:256]
        Y[1, 256 * j : 256 * (j + 1), :] = results[j]["y"][256:512]
    return Y


_NC_CACHE = {}


def _get_nc(debug=False):
    if debug not in _NC_CACHE:
        _NC_CACHE[debug] = build_nc(debug=debug)
    return _NC_CACHE[debug]


def kernel(hidden_states, attention_mask, Wq, bq, Wk, bk, Wv, bv, Wo, bo):
    # attention_mask is all-ones for this problem (spec: fill=ones) -> ignored
    nc = _get_nc(debug=False)
    in_maps = make_in_maps(hidden_states, Wq, bq, Wk, bk, Wv, bv, Wo, bo)
    res = run_bass_kernel_spmd(nc, in_maps, core_ids=list(range(N_CORES)))
    return assemble(res.results)


# revision 19
# speedup vs baseline: 1.2117x; 1.0464x over previous
"""Trainium2 Bass kernel for causal GQA self-attention (B=2, S=2048, H=2048,
16 heads / 4 KV heads, head_dim 128) on 8 NeuronCores.

Sharding: core i = (batch b = i//4, head-group g = i%4). Each core computes
QKV for heads 4g..4g+3 (= KV head g, no KV duplication) on its batch's 2048
rows, fused with flash-style attention per 512-row block as soon as that
block's QKV lands. Two 8-way AllToAlls then switch to row-sharding: core j
computes rows 256j..256j+256 of BOTH batches through the output projection
with the full SBUF-resident Wo (each 512-row attention block splits its output
between two destination cores, so the 8-way exchange is fully utilized).

Per-core dataflow: XT[h, r] (bf16, host-pretransposed, batch slice) -> QT/KT
channel-major via weight-stationary matmuls (per-output kt-loops, 2 rotating
PSUM banks); V directly in [k, d] layout via X-stationary matmuls (bias as a
rank-1 ones matmul). scores S^T[k, q] = KT_tile.T @ QT; causal via
compile-time tile skipping + one [128,128] triangle mask added on GpSimd; exp
on ScalarE; AV and a 128-row column-sum matmul (ones stationary - its output
IS the broadcast denominator) accumulate in PSUM; normalize with
reciprocal_approx_fast + one vector multiply. Four late attention blocks are
held back to overlap the first collective.

The attention_mask input is all-ones for this problem (spec fill=ones), so it
is ignored. All matmuls take bf16 inputs (fp32 PSUM accumulate).
"""

import sys

sys.path.insert(0, "/opt/trn_rl_repo")

from contextlib import ExitStack

import numpy as np
import ml_dtypes

import concourse.bass as bass
import concourse.mybir as mybir
import concourse.tile as tile
from concourse import bacc
from concourse.bass_utils import run_bass_kernel_spmd

F32 = mybir.dt.float32
BF16 = mybir.dt.bfloat16
AF = mybir.ActivationFunctionType

N_CORES = 8
B, S, HID = 2, 2048, 2048
NH, NKV, D = 16, 4, 128
SCALE = 1.0 / np.sqrt(D)
NEG = -1e30
P = 128
N_KT = HID // P  # 16 contraction tiles
N_QB = S // 512  # 4 row blocks per core (one batch)
HELD = {(2, 1), (3, 1), (2, 2), (3, 2), (2, 3), (3, 3)}  # run under A2A-1


def build_nc(debug=False):
    nc = bacc.Bacc("TRN2", target_bir_lowering=False, debug=debug, num_devices=8)

    # host-prepacked so every DMA line is >=2KB per partition:
    # xt[p, qb, t, r] ; wq[p, t, c] ; wk/wv[p, t*c]
    xt = nc.dram_tensor("xt", [P, N_QB, N_KT, 512], BF16, kind="ExternalInput")
    wq = nc.dram_tensor("wq", [P, N_KT, 512], BF16, kind="ExternalInput")
    wk = nc.dram_tensor("wk", [P, N_KT * 128], BF16, kind="ExternalInput")
    wv = nc.dram_tensor("wv", [P, N_KT * 128], BF16, kind="ExternalInput")
    bq = nc.dram_tensor("bq", [512, 1], F32, kind="ExternalInput")
    bk = nc.dram_tensor("bk", [128, 1], F32, kind="ExternalInput")
    bvr = nc.dram_tensor("bvr", [1, 128], BF16, kind="ExternalInput")
    wo = nc.dram_tensor("wo", [HID, HID], BF16, kind="ExternalInput")
    bo = nc.dram_tensor("bo", [1, HID], BF16, kind="ExternalInput")
    mtri = nc.dram_tensor("mtri", [P, P], BF16, kind="ExternalInput")
    onesd = nc.dram_tensor("onesd", [P, P], BF16, kind="ExternalInput")
    y = nc.dram_tensor("y", [512, HID], F32, kind="ExternalOutput")


    with tile.TileContext(nc) as tc, ExitStack() as top:
        persist = top.enter_context(tc.tile_pool(name="persist", bufs=1))
        dram = top.enter_context(tc.tile_pool(name="dram", bufs=1, space="DRAM"))

        a2a_in = [dram.tile([8, 2, P, 256], BF16, name=f"a2a_in{h}") for h in range(2)]
        a2a_out = [dram.tile([8, 2, P, 256], BF16, name=f"a2a_out{h}") for h in range(2)]

        wq_sb = persist.tile([P, N_KT, 512], BF16, tag="wq")
        wk_sb = persist.tile([P, N_KT, 128], BF16, tag="wk")
        wv_sb = persist.tile([P, N_KT, 128], BF16, tag="wv")

        # Small consts + the phase-4 Wo stream ride the gpsimd queue.
        ones_sq = persist.tile([P, P], BF16, tag="ones_sq")
        nc.gpsimd.dma_start(ones_sq[:], onesd[:])
        mtri_sb = persist.tile([P, P], BF16, tag="mtri")
        nc.gpsimd.dma_start(mtri_sb[:], mtri[:])
        bq_sb = persist.tile([P, 4], F32, tag="bq")
        for hh in range(4):
            nc.gpsimd.dma_start(bq_sb[:, hh : hh + 1], bq[128 * hh : 128 * (hh + 1), :])
        bk_sb = persist.tile([P, 1], F32, tag="bk")
        nc.gpsimd.dma_start(bk_sb[:], bk[:])
        bvr_sb = persist.tile([1, P], BF16, tag="bvr")
        nc.gpsimd.dma_start(bvr_sb[:], bvr[:])
        bo_sb = persist.tile([1, HID], BF16, tag="bo")
        nc.gpsimd.dma_start(bo_sb[:], bo[:])
        wo_sb = persist.tile([P, N_KT, HID], BF16, tag="wo")
        wo_loaded = [0]  # chunks streamed in during attention (HBM-idle window)

        # channel-major activations: partitions = feature dim
        qt_sb = persist.tile([P, 4, S], BF16, tag="qt")
        kt_sb = persist.tile([P, S], BF16, tag="kt")
        v_sb = persist.tile([P, N_KT, P], BF16, tag="v")  # [krow%128, ktile, d]

        with ExitStack() as body:
            xpool = body.enter_context(tc.tile_pool(name="xp", bufs=2))
            espool = body.enter_context(tc.tile_pool(name="es", bufs=6))
            bcpool = body.enter_context(tc.tile_pool(name="bc", bufs=2))
            aopool = body.enter_context(tc.tile_pool(name="ao", bufs=2))
            ps = body.enter_context(tc.tile_pool(name="ps", bufs=2, space="PSUM"))

            def attn_block(h, qb):
                h2, hj = h // 2, h % 2
                ktiles = list(range(4 * qb, 4 * qb + 4)) + list(range(4 * qb))
                ps_av = ps.tile([P, 512], F32, tag="av", name="ps_av")
                ps_cs = ps.tile([P, 512], F32, tag="cs", bufs=1, name="ps_cs")
                n_kt_q = len(ktiles)

                def emit_av(ki, q0, es, st, sp):
                    nc.tensor.matmul(
                        ps_av[:, q0:512], v_sb[:, ki, :],
                        es[:, q0:512], start=st, stop=sp,
                        skip_group_check=True,
                    )
                    nc.tensor.matmul(
                        ps_cs[:, q0:512], ones_sq[:, :],
                        es[:, q0:512], start=st, stop=sp,
                        skip_group_check=True,
                    )

                pending = []  # software-pipeline AV two k-tiles behind
                for idx, ki in enumerate(ktiles):
                    diag = ki >= 4 * qb
                    q0 = 128 * ki - 512 * qb if diag else 0
                    ps_s = ps.tile([P, 512], F32, tag="s", bufs=3, name="ps_s")
                    ksl = kt_sb[:, P * ki : P * (ki + 1)]
                    qsl = qt_sb[:, h, 512 * qb + q0 : 512 * (qb + 1)]
                    nc.tensor.matmul(ps_s[:, q0:512], ksl, qsl, start=True, stop=True)
                    es = espool.tile([P, 512], BF16, tag="es", name="es")
                    nc.scalar.activation(
                        es[:, q0:512], ps_s[:, q0:512], AF.Exp, scale=SCALE
                    )
                    if diag:
                        nc.vector.tensor_mul(
                            es[:, q0 : q0 + P], es[:, q0 : q0 + P], mtri_sb[:]
                        )
                    if len(pending) == 2:
                        emit_av(*pending.pop(0))
                    pending.append((ki, q0, es, idx == 0, idx == n_kt_q - 1))
                for args in pending:
                    emit_av(*args)

                bc = bcpool.tile([P, 512], F32, tag="bc", name="bc")
                nc.vector.reciprocal_approx_fast(out=bc[:], in_=ps_cs[:])
                ao = aopool.tile([P, 512], BF16, tag="ao", name="ao")
                nc.vector.tensor_mul(ao[:], ps_av[:], bc[:])
                nc.sync.dma_start(a2a_in[h2][2 * qb, hj, :, :], ao[:, 0:256])
                nc.sync.dma_start(a2a_in[h2][2 * qb + 1, hj, :, :], ao[:, 256:512])
                # 2 Wo chunks ride behind each early block's ao DMA: the ao's
                # wait on this block's output keeps the 8MB Wo stream out of
                # the phase-1 xt window (SWDGE DMAs have no data deps of their
                # own and would otherwise all fire at t=0)
                for _ in range(2):
                    if wo_loaded[0] < N_KT:
                        t = wo_loaded[0]
                        nc.sync.dma_start(wo_sb[:, t, :], wo[P * t : P * (t + 1), :])
                        wo_loaded[0] += 1

            # ---- fused QKV projection + attention, per 512-row block ----
            def load_xt(qb):
                xt_t = xpool.tile([P, N_KT, 512], BF16, tag="x", name="xt_t")
                for kc in range(4):
                    if qb == 0 and kc == 0:
                        nc.sync.dma_start(wk_sb[:].rearrange("p t c -> p (t c)"), wk[:, :])
                        nc.sync.dma_start(wv_sb[:].rearrange("p t c -> p (t c)"), wv[:, :])
                    nc.sync.dma_start(
                        xt_t[:, 4 * kc : 4 * (kc + 1), :],
                        xt[:, qb, 4 * kc : 4 * (kc + 1), :],
                    )
                    if qb == 0:
                        nc.sync.dma_start(
                            wq_sb[:, 4 * kc : 4 * kc + 4, :], wq[:, 4 * kc : 4 * kc + 4, :]
                        )
                return xt_t

            # HAM pre-warm: ~40 tiny matmuls keep the PE busy through its
            # 3.4us activity window while the first weights/xt stream in, so
            # the first real matmuls run at 2.4GHz instead of 1.2
            warm_ps = ps.tile([P, 512], F32, tag="p1", name="warm_ps")
            for wi in range(200):
                nc.tensor.matmul(
                    warm_ps[:, 0:128], ones_sq[:, :], ones_sq[:, :],
                    start=(wi == 0), stop=(wi == 199), skip_group_check=True,
                )
            xt_next = load_xt(0)
            for qb in range(N_QB):
                rsl = slice(512 * qb, 512 * (qb + 1))
                xt_t = xt_next
                # K: channel-major, weight-stationary
                ps_k = ps.tile([P, 512], F32, tag="p1", name="ps_k")
                for kt_i in range(N_KT):
                    nc.tensor.matmul(
                        ps_k[:], wk_sb[:, kt_i, :], xt_t[:, kt_i, :],
                        start=(kt_i == 0), stop=(kt_i == N_KT - 1),
                    )
                nc.vector.tensor_scalar_add(kt_sb[:, rsl], ps_k[:], bk_sb[:])
                # V: [k, d] layout, X-stationary; bias via rank-1 ones matmul
                ps_v = ps.tile([P, 4, P], F32, tag="p1", name="ps_v")
                for kb in range(4):
                    for kt_i in range(N_KT):
                        nc.tensor.matmul(
                            ps_v[:, kb, :],
                            xt_t[:, kt_i, P * kb : P * (kb + 1)],
                            wv_sb[:, kt_i, :],
                            start=(kt_i == 0), stop=False,
                            skip_group_check=True,
                        )
                    nc.tensor.matmul(
                        ps_v[:, kb, :], ones_sq[0:1, :], bvr_sb[:],
                        start=False, stop=True, skip_group_check=True,
                    )
                nc.vector.tensor_copy(v_sb[:, 4 * qb : 4 * qb + 4, :], ps_v[:])
                # Q per head, each head's attention block right behind it
                for hh in range(4):
                    ps_q = ps.tile([P, 512], F32, tag="p1", name="ps_q")
                    for kt_i in range(N_KT):
                        nc.tensor.matmul(
                            ps_q[:], wq_sb[:, kt_i, P * hh : P * (hh + 1)],
                            xt_t[:, kt_i, :],
                            start=(kt_i == 0), stop=(kt_i == N_KT - 1),
                        )
                    nc.vector.tensor_scalar_add(
                        qt_sb[:, hh, rsl], ps_q[:], bq_sb[:, hh : hh + 1]
                    )
                    if hh == 0 and qb + 1 < N_QB:
                        # prefetch the next row block before this qb's ao DMAs
                        # can head-of-line-block the sync queue
                        xt_next = load_xt(qb + 1)
                    if (hh, qb) not in HELD:
                        attn_block(hh, qb)

            # ---- A2A for heads {0,1} of each peer; held blocks run under it
            nc.gpsimd.collective_compute(
                "AllToAll",
                mybir.AluOpType.bypass,
                replica_groups=[list(range(N_CORES))],
                ins=[a2a_in[0][:]],
                outs=[a2a_out[0][:]],
            )
            for h, qb in sorted(HELD, key=lambda t: (t[1], t[0])):
                attn_block(h, qb)
            nc.gpsimd.collective_compute(
                "AllToAll",
                mybir.AluOpType.bypass,
                replica_groups=[list(range(N_CORES))],
                ins=[a2a_in[1][:]],
                outs=[a2a_out[1][:]],
            )

        # ---- o_proj (512 rows x 2048, SBUF-resident Wo) ----
        # hd-tile t = head t (channels 128t..); t%4 in {0,1} arrives with
        # A2A-1, {2,3} with A2A-2.
        with ExitStack() as ph4:
            atpool = ph4.enter_context(tc.tile_pool(name="at", bufs=1))
            ypool = ph4.enter_context(tc.tile_pool(name="yp", bufs=4))
            yppool = ph4.enter_context(tc.tile_pool(name="ypart", bufs=1))
            pso = ph4.enter_context(tc.tile_pool(name="pso", bufs=8, space="PSUM"))
            pass1 = [t for t in range(N_KT) if t % 4 < 2]
            pass2 = [t for t in range(N_KT) if t % 4 >= 2]
            # head t = 4g + 2*h2 + hj comes from cores g (batch-0 row half)
            # and 4+g (batch-1 half). One DMA per A2A half: dst free dims
            # (g, b, hj, c) <- src dims (b, g, hj, p, c).
            at_all = [None, None]
            for h2 in range(2):
                a = atpool.tile([P, 4, 2, 2, 256], BF16, tag=f"atall{h2}", name=f"atall{h2}")
                for bb in range(2):
                    nc.gpsimd.dma_start(
                        a[:, :, :, bb, :],
                        a2a_out[h2][4 * bb : 4 * bb + 4, :, :, :].rearrange(
                            "g hj p c -> p g hj c"
                        ),
                    )
                at_all[h2] = a
            # at[t] view [p, 512]: head t rows = [b0 256 | b1 256]
            at = [
                at_all[(t % 4) // 2][:, t // 4, t % 2, :, :].rearrange("p b c -> p (b c)")
                for t in range(N_KT)
            ]
            ypart = [[None] * 4 for _ in range(4)]
            for nbp in range(2):  # nb pairs share each stationary LDWEIGHTS
                nbs = (2 * nbp, 2 * nbp + 1)
                ps_os = {
                    (nb, q): pso.tile([P, 512], F32, tag="po", name=f"ps_o{nb}_{q}")
                    for nb in nbs for q in range(4)
                }
                for ti, t in enumerate(pass1):
                    for qt_i in range(4):
                        for nb in nbs:
                            nc.tensor.matmul(
                                ps_os[nb, qt_i][:], at[t][:, P * qt_i : P * (qt_i + 1)],
                                wo_sb[:, t, 512 * nb : 512 * (nb + 1)],
                                start=(ti == 0), stop=False,
                                skip_group_check=True,
                            )
                for nb in nbs:
                    for qt_i in range(4):
                        nc.tensor.matmul(
                            ps_os[nb, qt_i][:], ones_sq[0:1, :],
                            bo_sb[0:1, 512 * nb : 512 * (nb + 1)], start=False, stop=True,
                            skip_group_check=True,
                        )
                        yp = yppool.tile([P, 512], F32, tag=f"yp{nb}_{qt_i}", name="yp")
                        nc.vector.tensor_copy(yp[:], ps_os[nb, qt_i][:])
                        ypart[nb][qt_i] = yp
            for nbp in range(2):
                nbs = (2 * nbp, 2 * nbp + 1)
                last = nbp == 1
                if not last:
                    ps_o2 = {
                        (nb, q): pso.tile([P, 512], F32, tag="po", name=f"ps_p{nb}_{q}")
                        for nb in nbs for q in range(4)
                    }
                    for ti, t in enumerate(pass2):
                        for qt_i in range(4):
                            for nb in nbs:
                                nc.tensor.matmul(
                                    ps_o2[nb, qt_i][:], at[t][:, P * qt_i : P * (qt_i + 1)],
                                    wo_sb[:, t, 512 * nb : 512 * (nb + 1)],
                                    start=(ti == 0), stop=(ti == len(pass2) - 1),
                                    skip_group_check=True,
                                )
                    for nb in nbs:
                        nsl = slice(512 * nb, 512 * (nb + 1))
                        for qt_i in range(4):
                            ysb = ypool.tile([P, 512], F32, tag="y", name="ysb")
                            nc.vector.tensor_add(ysb[:], ps_o2[nb, qt_i][:], ypart[nb][qt_i][:])
                            nc.sync.dma_start(y[P * qt_i : P * (qt_i + 1), nsl], ysb[:])
                else:
                    # last pair: (nb, qt) outer so each bank finishes early and
                    # its eviction + y DMA overlap the remaining matmuls
                    for nb in nbs:
                        nsl = slice(512 * nb, 512 * (nb + 1))
                        for qt_i in range(4):
                            ps_p = pso.tile([P, 512], F32, tag="po", name="ps_p")
                            for ti, t in enumerate(pass2):
                                nc.tensor.matmul(
                                    ps_p[:], at[t][:, P * qt_i : P * (qt_i + 1)],
                                    wo_sb[:, t, 512 * nb : 512 * (nb + 1)],
                                    start=(ti == 0), stop=(ti == len(pass2) - 1),
                                    skip_group_check=True,
                                )
                            ysb = ypool.tile([P, 512], F32, tag="y", name="ysb")
                            nc.vector.tensor_add(ysb[:], ps_p[:], ypart[nb][qt_i][:])
                            nc.sync.dma_start(y[P * qt_i : P * (qt_i + 1), nsl], ysb[:])

    nc.compile()
    return nc


def make_in_maps(hidden_states, Wq, bq, Wk, bk, Wv, bv, Wo, bo):
    X = np.asarray(hidden_states, np.float32)  # [B, S, HID]
    qq = np.arange(P)[None, :]
    kk = np.arange(P)[:, None]
    mtri = np.where(qq >= kk, 1.0, 0.0).astype(ml_dtypes.bfloat16)
    Wq = np.asarray(Wq, np.float32)
    Wk = np.asarray(Wk, np.float32)
    Wv = np.asarray(Wv, np.float32)
    Wo = np.ascontiguousarray(np.asarray(Wo, np.float32)).astype(ml_dtypes.bfloat16)
    bq = np.asarray(bq, np.float32)
    bk = np.asarray(bk, np.float32)
    bv = np.asarray(bv, np.float32)
    bo = np.asarray(bo, np.float32)
    def pack_w(W):  # [HID, C] -> [128, N_KT, C] (t = hid tile)
        return np.ascontiguousarray(
            W.reshape(N_KT, P, -1).transpose(1, 0, 2)
        ).astype(ml_dtypes.bfloat16)

    xts = []
    for b in range(B):
        XT = X[b].T.reshape(N_KT, P, N_QB, 512)  # [t, p, qb, r]
        xts.append(
            np.ascontiguousarray(XT.transpose(1, 2, 0, 3)).astype(ml_dtypes.bfloat16)
        )
    in_maps = []
    for i in range(N_CORES):
        b, g = i // 4, i % 4
        in_maps.append({
            "xt": xts[b],
            "wq": pack_w(Wq[:, 512 * g : 512 * (g + 1)]),
            "wk": pack_w(Wk[:, 128 * g : 128 * (g + 1)]).reshape(P, N_KT * 128),
            "wv": pack_w(Wv[:, 128 * g : 128 * (g + 1)]).reshape(P, N_KT * 128),
            "bq": np.ascontiguousarray(bq[512 * g : 512 * (g + 1)]).reshape(512, 1),
            "bk": np.ascontiguousarray(bk[128 * g : 128 * (g + 1)]).reshape(128, 1),
            "bvr": np.ascontiguousarray(bv[128 * g : 128 * (g + 1)]).reshape(1, 128).astype(ml_dtypes.bfloat16),
            "wo": Wo,
            "bo": bo.reshape(1, HID).astype(ml_dtypes.bfloat16),
            "mtri": mtri,
            "onesd": np.ones((P, P), ml_dtypes.bfloat16),
        })
    return in_maps


def assemble(results):
    Y = np.empty((B, S, HID), np.float32)
    for j in range(N_CORES):
        Y[0, 256 * j : 256 * (j + 1), :] = results[j]["y"][0:256]
        Y[1, 256 * j : 256 * (j + 1), :] = results[j]["y"][256:512]
    return Y


_NC_CACHE = {}


def _get_nc(debug=False):
    if debug not in _NC_CACHE:
        _NC_CACHE[debug] = build_nc(debug=debug)
    return _NC_CACHE[debug]


def kernel(hidden_states, attention_mask, Wq, bq, Wk, bk, Wv, bv, Wo, bo):
    # attention_mask is all-ones for this problem (spec: fill=ones) -> ignored
    nc = _get_nc(debug=False)
    in_maps = make_in_maps(hidden_states, Wq, bq, Wk, bk, Wv, bv, Wo, bo)
    res = run_bass_kernel_spmd(nc, in_maps, core_ids=list(range(N_CORES)))
    return assemble(res.results)
